# revision 34
# baseline (speedup 1.0000x reference)
"""MoE grouped linear (DMoELinear) on 8 Trainium2 NeuronCores.

Expert-parallel sharding: tokens are sorted by expert id, so expert e's
tokens form one contiguous slice. Core e receives expert e's tokens
(padded to a uniform capacity C = max group size, so all cores run one
SPMD NEFF), expert e's weight and bias, and computes
    yT_e = (x_e @ W_e.T).T.bf16 + b_e.bf16
with the weight block as the stationary matmul operand and tokens as
the moving free dim.

v2 schedule (trace-driven): the run is paced by DMA in the first third
and by the PE after, with a fixed ~8us NEFF epilogue. So: few, large
DMAs (each dma_start costs ~0.6us of issuing-engine time regardless of
size), deadline-ordered across the two HWDGE rings; weights flat
[128, DB*KT*128] so multi-block transfers are single 2D slices; w0/w1
split into k-slices only where the PE needs them early; remaining
weights as 2-block 512KB packs; x as 8 full-width k-tiles alternating
rings. db0/db1 interleave k-steps during the x trickle. Bias add fuses
into PSUM eviction (ACT/DVE alternating).
"""

import numpy as np
import ml_dtypes

N_TOK, D_IN, D_OUT, N_EXP = 8192, 1024, 2048, 8
N_CORES = 8
P = 128
NFREE = 512  # max matmul moving free dim (one PSUM bank of f32)

BF16 = ml_dtypes.bfloat16

_nc_cache: dict[int, object] = {}


def _chunks(C):
    out = []
    off = 0
    while off < C:
        cw = min(NFREE, C - off)
        out.append((off, cw))
        off += cw
    return out


def _build_bass(C: int):
    """Emit the per-core Bass/Tile kernel for token capacity C."""
    import concourse.bass as bass  # noqa: F401  (registers engines)
    import concourse.mybir as mybir
    import concourse.tile as tile
    from concourse import bacc

    dt = mybir.dt
    KT = D_IN // P      # 8 contraction tiles
    DB = D_OUT // P     # 16 output-row blocks
    KW = KT * P         # columns per db block in the flat weight (1024)
    chunks = _chunks(C)
    chunk_of_db = {db: chunks for db in range(DB)}

    nc = bacc.Bacc("TRN2", target_bir_lowering=False)

    # x partition-flat: row p, col ki*C + c  =  x[token c, ki*128+p]
    # (k-tile pairs are contiguous 2D slices → one DMA each).
    xf_d = nc.dram_tensor("xf", [P, KT * C], dt.bfloat16, kind="ExternalInput")
    # flat weights: row p, col db*1024 + kt*128 + d  (lhsT slices are
    # contiguous 128-col blocks; multi-db packs are contiguous too).
    wf_d = nc.dram_tensor("wf", [P, DB * KW], dt.bfloat16, kind="ExternalInput")
    bias_d = nc.dram_tensor("biasp", [P, DB], dt.float32, kind="ExternalInput")
    y_d = nc.dram_tensor("yT", [D_OUT, C], dt.bfloat16, kind="ExternalOutput")

    with tile.TileContext(nc) as tc:
        with (
            tc.tile_pool(name="persist", bufs=1) as ppool,
            tc.tile_pool(name="yout", bufs=4) as ypool,
            tc.tile_pool(name="psum", bufs=8, space="PSUM") as pspool,
        ):
            # x as single 277KB k-tile transfers: pair-sized transfers
            # stream ~25% faster but their 1.5-2us arrival lumps starve
            # the PE long enough to risk a HAM re-throttle (measured:
            # one re-throttle costs 3-8us; the rate win is ~1us).
            x_tiles = [
                ppool.tile([P, C], dt.bfloat16, name=f"x{ki}", tag=f"x{ki}")
                for ki in range(KT)
            ]

            def x_sl(ki, off, cw):
                return x_tiles[ki][:, off:off + cw]
            w_s = [
                ppool.tile([P, KW], dt.bfloat16, name=f"w{db}", tag=f"w{db}")
                for db in range(4)
            ]
            packs = [
                ppool.tile([P, 2 * KW], dt.bfloat16, name=f"wp{g}", tag=f"wp{g}")
                for g in range(2, 8)
            ]
            bt = ppool.tile([P, DB], dt.float32, name="bias", tag="bias")

            def lhsT(db, ki):
                if db < 4:
                    return w_s[db][:, ki * P:(ki + 1) * P]
                g = db // 2
                off = (db - 2 * g) * KW + ki * P
                return packs[g - 2][:, off:off + P]

            # ── DMA schedule: two HWDGE rings, deadline-ordered ──────
            # ring A = sync, ring B = scalar. db1 is staggered 4 k-steps
            # behind db0, so during the db0-only phase both rings carry
            # almost pure x at full rate; w1's big slices, w2/w3 and the
            # packs all ride during db1's catch-up / db2's run where
            # deadlines have slack.
            A, B = nc.sync, nc.scalar

            def wsl(db, k0, k1):
                eng = A if db % 2 == 0 else B
                eng.dma_start(
                    w_s[db][:, k0 * P:k1 * P],
                    wf_d[:, db * KW + k0 * P:db * KW + k1 * P],
                )

            def xdma(ki, eng):
                eng.dma_start(x_tiles[ki][:], xf_d[:, ki * C:(ki + 1) * C])

            # Deadlines assume db0/db1 interleave with a 1-step stagger:
            # step i needs x_i, w0k_i AND w1k_{i-1}, so both dbs' k-slice
            # pairs ride just ahead of their step, w1's first slice
            # leading ring B.
            wsl(0, 0, 2)       # A
            wsl(1, 0, 2)       # B
            xdma(0, B)
            xdma(1, A)
            xdma(2, B)
            xdma(3, A)
            wsl(0, 2, 4)       # A
            wsl(1, 2, 4)       # B
            xdma(4, B)
            xdma(5, A)
            wsl(0, 4, 8)       # A
            wsl(1, 4, 8)       # B
            xdma(6, B)
            xdma(7, A)
            B.dma_start(bt[:], bias_d[:])
            A.dma_start(w_s[2][:], wf_d[:, 2 * KW:3 * KW])
            A.dma_start(w_s[3][:], wf_d[:, 3 * KW:4 * KW])
            for g in range(2, 8):
                eng = B if g % 2 == 0 else A
                eng.dma_start(packs[g - 2][:], wf_d[:, 2 * g * KW:(2 * g + 2) * KW])

            # ── PE warmup: flip the HAM clock gate (~3.4us of activity)
            # while the first DMAs land.
            warm = ppool.tile([P, P], dt.bfloat16, name="warm", tag="warm")
            nc.vector.memset(warm[:], 0.0)
            wps = pspool.tile([P, P], dt.float32, name="wps", tag="ps")

            def warm_mm(n):
                for _ in range(n):
                    nc.tensor.matmul(wps[:], warm[:], warm[:], start=True, stop=True)

            # Long enough that the HAM clock-gate flip (one full 4096-
            # cycle busy window + phase, worst ~4.5us) always lands
            # inside the warmup: the flip arriving mid-real-work costs
            # 3-8us of half-clock matmuls, far worse than the ~1us of
            # extra warmup. The x stream is still arriving anyway.
            warm_mm(40)

            all_psums = {}

            def alloc_chunk(db, j):
                _, cw = chunk_of_db[db][j]
                return pspool.tile([P, cw], dt.float32, name=f"ps{db}_{j}", tag="ps")

            def alloc_psums(db, chunks_j=None):
                js = chunks_j or range(len(chunk_of_db[db]))
                cur = all_psums.setdefault(db, {})
                for j in js:
                    cur[j] = alloc_chunk(db, j)

            def emit_mm(db, ki, j):
                off, cw = chunk_of_db[db][j]
                nc.tensor.matmul(
                    all_psums[db][j][:, :cw],
                    lhsT(db, ki),
                    x_sl(ki, off, cw),
                    start=(ki == 0),
                    stop=(ki == KT - 1),
                )

            def emit_mms(db, ki, chunks_j=None):
                for j in chunks_j or range(len(chunk_of_db[db])):
                    emit_mm(db, ki, j)

            ep = 0
            ysbs = {}

            def new_ysb(db):
                ysbs[db] = ypool.tile([P, C], dt.bfloat16, name="ysb", tag="ysb")
                return ysbs[db]

            def evict_chunk(db, j, ysb):
                nonlocal ep
                off, cw = chunk_of_db[db][j]
                bias_col = bt[:, db:db + 1]
                if ep % 2 == 0:
                    nc.scalar.add(ysb[:, off:off + cw], all_psums[db][j][:, :cw], bias_col)
                else:
                    nc.vector.tensor_scalar_add(
                        ysb[:, off:off + cw], all_psums[db][j][:, :cw], bias_col
                    )
                ep += 1

            def evict(db):
                ysb = new_ysb(db)
                for j in range(len(chunk_of_db[db])):
                    evict_chunk(db, j, ysb)
                return ysb

            def ydma(db, ysb):
                eng = nc.sync if db % 2 == 0 else nc.scalar
                eng.dma_start(y_d[db * P:(db + 1) * P, :], ysb[:])

            # ── Trickle phase ────────────────────────────────────────
            # db0/db1 interleaved by k-step; db1 one step behind so
            # db0's k7 chunks finish (and their PSUM banks evict) while
            # db1's tail runs — db2 then starts without a bank wait.
            STAG = 1
            alloc_psums(0)
            alloc_psums(1)
            for step in range(KT + STAG):
                if step < KT:
                    emit_mms(0, step)
                if step >= STAG:
                    emit_mms(1, step - STAG)
            ydma(0, evict(0))
            ydma(1, evict(1))

            for db in range(2, DB):
                alloc_psums(db)
                last = db == DB - 1
                n_ch = len(chunk_of_db[db])
                for ki in range(KT):
                    if last and ki == KT - 1 and n_ch == 3:
                        # drain the small chunk first: its eviction+DMA
                        # start while the big chunks' last matmuls run
                        emit_mms(db, ki, [2, 0, 1])
                    else:
                        emit_mms(db, ki)
                if last:
                    # final block drains the kernel: small chunk evicted
                    # on DVE immediately, big chunks concurrently on
                    # ACT+DVE; chunk DMAs split so scalar only issues
                    # one (its queue also runs the ACT eviction).
                    psums = all_psums[db]
                    ysb = ypool.tile([P, C], dt.bfloat16, name="ysb", tag="ysb")
                    bias_col = bt[:, db:db + 1]
                    cks = chunk_of_db[db]
                    for j in ([2, 0, 1] if n_ch == 3 else range(n_ch)):
                        off, cw = cks[j]
                        if j == 0:
                            nc.scalar.add(ysb[:, off:off + cw], psums[j][:, :cw], bias_col)
                        else:
                            nc.vector.tensor_scalar_add(
                                ysb[:, off:off + cw], psums[j][:, :cw], bias_col
                            )
                    for j in ([2, 0, 1] if n_ch == 3 else range(n_ch)):
                        off, cw = cks[j]
                        eng = nc.scalar if j == 1 else nc.sync
                        eng.dma_start(
                            y_d[db * P:(db + 1) * P, off:off + cw],
                            ysb[:, off:off + cw],
                        )
                elif db == DB - 2:
                    # per-chunk DMAs on both rings so the tail pipelines
                    ysb = evict(db)
                    for j, (off, cw) in enumerate(chunk_of_db[db]):
                        eng = nc.sync if (db + j) % 2 == 0 else nc.scalar
                        eng.dma_start(
                            y_d[db * P:(db + 1) * P, off:off + cw],
                            ysb[:, off:off + cw],
                        )
                else:
                    ydma(db, evict(db))

    nc.compile()
    return nc


def _run_spmd(in_maps, C, trace=False, trace_cores=None):
    from concourse.bass_utils import run_bass_kernel_spmd

    nc = _nc_cache.get(C)
    if nc is None:
        nc = _build_bass(C)
        _nc_cache[C] = nc
    return run_bass_kernel_spmd(
        nc,
        in_maps,
        core_ids=list(range(N_CORES)),
        trace=trace,
        trace_cores=trace_cores,
    )


def _prepare(x, weight, bias, ids_sorted):
    """Host-side routing: returns (in_maps, C, counts, starts)."""
    x = np.asarray(x)
    weight = np.asarray(weight)
    bias = np.asarray(bias)
    ids = np.asarray(ids_sorted)

    counts = np.bincount(ids, minlength=N_EXP).astype(np.int64)
    starts = np.zeros(N_EXP, dtype=np.int64)
    starts[1:] = np.cumsum(counts)[:-1]
    C = max(int(counts.max()), 2)
    C += C % 2

    KT = D_IN // P
    DB = D_OUT // P
    xb = x.astype(BF16)
    in_maps = []
    for e in range(N_EXP):
        n_e = int(counts[e])
        xeT = np.zeros((D_IN, C), dtype=BF16)
        if n_e:
            xeT[:, :n_e] = xb[starts[e]:starts[e] + n_e].T
        # partition-flat x: row p, col ki*C + c = x[token c, ki*128+p]
        xf = np.ascontiguousarray(
            xeT.reshape(KT, P, C).transpose(1, 0, 2)
        ).reshape(P, KT * C)
        # flat weight: row p, col db*1024 + kt*128 + d  = W_e[db*128+d, kt*128+p]
        weT = weight[e].T.astype(BF16)  # [d_in, d_out]
        wf = np.ascontiguousarray(
            weT.reshape(KT, P, DB, P).transpose(1, 2, 0, 3)
        ).reshape(P, DB * KT * P)
        bp = np.ascontiguousarray(
            bias[e].astype(BF16).astype(np.float32).reshape(DB, P).T
        )
        in_maps.append({"xf": xf, "wf": wf, "biasp": bp})
    return in_maps, C, counts, starts


def _assemble(results, counts, starts):
    out = np.empty((N_TOK, D_OUT), dtype=BF16)
    for e in range(N_EXP):
        n_e = int(counts[e])
        if n_e:
            out[starts[e]:starts[e] + n_e] = results[e]["yT"][:, :n_e].T
    return out


def kernel(x, weight, bias, ids_sorted):
    in_maps, C, counts, starts = _prepare(x, weight, bias, ids_sorted)
    res = _run_spmd(in_maps, C)
    return _assemble(res.results, counts, starts)


# revision 35
# speedup vs baseline: 1.0061x; 1.0061x over previous
"""MoE grouped linear (DMoELinear) on 8 Trainium2 NeuronCores.

Expert-parallel sharding: tokens are sorted by expert id, so expert e's
tokens form one contiguous slice. Core e receives expert e's tokens
(padded to a uniform capacity C = max group size, so all cores run one
SPMD NEFF), expert e's weight and bias, and computes
    yT_e = (x_e @ W_e.T).T.bf16 + b_e.bf16
with the weight block as the stationary matmul operand and tokens as
the moving free dim.

v2 schedule (trace-driven): the run is paced by DMA in the first third
and by the PE after, with a fixed ~8us NEFF epilogue. So: few, large
DMAs (each dma_start costs ~0.6us of issuing-engine time regardless of
size), deadline-ordered across the two HWDGE rings; weights flat
[128, DB*KT*128] so multi-block transfers are single 2D slices; w0/w1
split into k-slices only where the PE needs them early; remaining
weights as 2-block 512KB packs; x as 8 full-width k-tiles alternating
rings. db0/db1 interleave k-steps during the x trickle. Bias add fuses
into PSUM eviction (ACT/DVE alternating).
"""

import numpy as np
import ml_dtypes

N_TOK, D_IN, D_OUT, N_EXP = 8192, 1024, 2048, 8
N_CORES = 8
P = 128
NFREE = 512  # max matmul moving free dim (one PSUM bank of f32)

BF16 = ml_dtypes.bfloat16

_nc_cache: dict[int, object] = {}


def _chunks(C):
    out = []
    off = 0
    while off < C:
        cw = min(NFREE, C - off)
        out.append((off, cw))
        off += cw
    return out


def _build_bass(C: int):
    """Emit the per-core Bass/Tile kernel for token capacity C."""
    import concourse.bass as bass  # noqa: F401  (registers engines)
    import concourse.mybir as mybir
    import concourse.tile as tile
    from concourse import bacc

    dt = mybir.dt
    KT = D_IN // P      # 8 contraction tiles
    DB = D_OUT // P     # 16 output-row blocks
    KW = KT * P         # columns per db block in the flat weight (1024)
    chunks = _chunks(C)
    chunk_of_db = {db: chunks for db in range(DB)}

    nc = bacc.Bacc("TRN2", target_bir_lowering=False)

    # x partition-flat: row p, col ki*C + c  =  x[token c, ki*128+p]
    # (k-tile pairs are contiguous 2D slices → one DMA each).
    xf_d = nc.dram_tensor("xf", [P, KT * C], dt.bfloat16, kind="ExternalInput")
    # flat weights: row p, col db*1024 + kt*128 + d  (lhsT slices are
    # contiguous 128-col blocks; multi-db packs are contiguous too).
    wf_d = nc.dram_tensor("wf", [P, DB * KW], dt.bfloat16, kind="ExternalInput")
    bias_d = nc.dram_tensor("biasp", [P, DB], dt.float32, kind="ExternalInput")
    y_d = nc.dram_tensor("yT", [D_OUT, C], dt.bfloat16, kind="ExternalOutput")

    with tile.TileContext(nc) as tc:
        with (
            tc.tile_pool(name="persist", bufs=1) as ppool,
            tc.tile_pool(name="yout", bufs=4) as ypool,
            tc.tile_pool(name="psum", bufs=8, space="PSUM") as pspool,
        ):
            # x as single 277KB k-tile transfers: pair-sized transfers
            # stream ~25% faster but their 1.5-2us arrival lumps starve
            # the PE long enough to risk a HAM re-throttle (measured:
            # one re-throttle costs 3-8us; the rate win is ~1us).
            x_tiles = [
                ppool.tile([P, C], dt.bfloat16, name=f"x{ki}", tag=f"x{ki}")
                for ki in range(KT)
            ]

            def x_sl(ki, off, cw):
                return x_tiles[ki][:, off:off + cw]
            w_s = [
                ppool.tile([P, KW], dt.bfloat16, name=f"w{db}", tag=f"w{db}")
                for db in range(4)
            ]
            packs = [
                ppool.tile([P, 2 * KW], dt.bfloat16, name=f"wp{g}", tag=f"wp{g}")
                for g in range(2, 8)
            ]
            bt = ppool.tile([P, DB], dt.float32, name="bias", tag="bias")

            def lhsT(db, ki):
                if db < 4:
                    return w_s[db][:, ki * P:(ki + 1) * P]
                g = db // 2
                off = (db - 2 * g) * KW + ki * P
                return packs[g - 2][:, off:off + P]

            # ── DMA schedule: two HWDGE rings, deadline-ordered ──────
            # ring A = sync, ring B = scalar. db1 is staggered 4 k-steps
            # behind db0, so during the db0-only phase both rings carry
            # almost pure x at full rate; w1's big slices, w2/w3 and the
            # packs all ride during db1's catch-up / db2's run where
            # deadlines have slack.
            A, B = nc.sync, nc.scalar

            def wsl(db, k0, k1):
                eng = A if db % 2 == 0 else B
                eng.dma_start(
                    w_s[db][:, k0 * P:k1 * P],
                    wf_d[:, db * KW + k0 * P:db * KW + k1 * P],
                )

            def xdma(ki, eng):
                eng.dma_start(x_tiles[ki][:], xf_d[:, ki * C:(ki + 1) * C])

            # Deadlines assume db0/db1 interleave with a 1-step stagger:
            # step i needs x_i, w0k_i AND w1k_{i-1}, so both dbs' k-slice
            # pairs ride just ahead of their step, w1's first slice
            # leading ring B.
            wsl(0, 0, 2)       # A
            wsl(1, 0, 2)       # B
            xdma(0, B)
            xdma(1, A)
            xdma(2, B)
            xdma(3, A)
            wsl(0, 2, 4)       # A
            wsl(1, 2, 4)       # B
            xdma(4, B)
            xdma(5, A)
            wsl(0, 4, 8)       # A
            wsl(1, 4, 8)       # B
            xdma(6, B)
            xdma(7, A)
            B.dma_start(bt[:], bias_d[:])
            A.dma_start(w_s[2][:], wf_d[:, 2 * KW:3 * KW])
            A.dma_start(w_s[3][:], wf_d[:, 3 * KW:4 * KW])
            for g in range(2, 8):
                eng = B if g % 2 == 0 else A
                eng.dma_start(packs[g - 2][:], wf_d[:, 2 * g * KW:(2 * g + 2) * KW])

            # ── PE warmup: flip the HAM clock gate (~3.4us of activity)
            # while the first DMAs land.
            warm = ppool.tile([P, P], dt.bfloat16, name="warm", tag="warm")
            nc.vector.memset(warm[:], 0.0)
            wps = pspool.tile([P, P], dt.float32, name="wps", tag="ps")

            def warm_mm(n):
                for _ in range(n):
                    nc.tensor.matmul(wps[:], warm[:], warm[:], start=True, stop=True)

            # Long enough that the HAM clock-gate flip (one full 4096-
            # cycle busy window + phase, worst ~4.5us) always lands
            # inside the warmup: the flip arriving mid-real-work costs
            # 3-8us of half-clock matmuls, far worse than the ~1us of
            # extra warmup. The x stream is still arriving anyway.
            warm_mm(40)

            all_psums = {}

            def alloc_chunk(db, j):
                _, cw = chunk_of_db[db][j]
                return pspool.tile([P, cw], dt.float32, name=f"ps{db}_{j}", tag="ps")

            def alloc_psums(db, chunks_j=None):
                js = chunks_j or range(len(chunk_of_db[db]))
                cur = all_psums.setdefault(db, {})
                for j in js:
                    cur[j] = alloc_chunk(db, j)

            def emit_mm(db, ki, j):
                off, cw = chunk_of_db[db][j]
                nc.tensor.matmul(
                    all_psums[db][j][:, :cw],
                    lhsT(db, ki),
                    x_sl(ki, off, cw),
                    start=(ki == 0),
                    stop=(ki == KT - 1),
                )

            def emit_mms(db, ki, chunks_j=None):
                for j in chunks_j or range(len(chunk_of_db[db])):
                    emit_mm(db, ki, j)

            ep = 0
            ysbs = {}

            def new_ysb(db):
                ysbs[db] = ypool.tile([P, C], dt.bfloat16, name="ysb", tag="ysb")
                return ysbs[db]

            def evict_chunk(db, j, ysb):
                nonlocal ep
                off, cw = chunk_of_db[db][j]
                bias_col = bt[:, db:db + 1]
                if ep % 2 == 0:
                    nc.scalar.add(ysb[:, off:off + cw], all_psums[db][j][:, :cw], bias_col)
                else:
                    nc.vector.tensor_scalar_add(
                        ysb[:, off:off + cw], all_psums[db][j][:, :cw], bias_col
                    )
                ep += 1

            def evict(db):
                ysb = new_ysb(db)
                for j in range(len(chunk_of_db[db])):
                    evict_chunk(db, j, ysb)
                return ysb

            def ydma(db, ysb):
                eng = nc.sync if db % 2 == 0 else nc.scalar
                eng.dma_start(y_d[db * P:(db + 1) * P, :], ysb[:])

            # ── Trickle phase ────────────────────────────────────────
            # db0/db1 interleaved by k-step; db1 one step behind so
            # db0's k7 chunks finish (and their PSUM banks evict) while
            # db1's tail runs — db2 then starts without a bank wait.
            STAG = 1
            alloc_psums(0)
            alloc_psums(1)
            for step in range(KT + STAG):
                if step < KT:
                    emit_mms(0, step)
                if step >= STAG:
                    emit_mms(1, step - STAG)
            ydma(0, evict(0))
            ydma(1, evict(1))

            for db in range(2, DB):
                alloc_psums(db)
                for ki in range(KT):
                    emit_mms(db, ki)
                if db == DB - 1:
                    # final block drains the kernel: big chunks evicted
                    # concurrently on ACT+DVE, the small chunk on DVE,
                    # chunk DMAs split so scalar only issues one (its
                    # queue also runs the ACT eviction).
                    psums = all_psums[db]
                    ysb = ypool.tile([P, C], dt.bfloat16, name="ysb", tag="ysb")
                    bias_col = bt[:, db:db + 1]
                    cks = chunk_of_db[db]
                    o0, w0_ = cks[0]
                    nc.scalar.add(ysb[:, o0:o0 + w0_], psums[0][:, :w0_], bias_col)
                    for j, (off, cw) in list(enumerate(cks))[1:]:
                        nc.vector.tensor_scalar_add(
                            ysb[:, off:off + cw], psums[j][:, :cw], bias_col
                        )
                    for j, (off, cw) in enumerate(cks):
                        eng = nc.scalar if j == 1 else nc.sync
                        eng.dma_start(
                            y_d[db * P:(db + 1) * P, off:off + cw],
                            ysb[:, off:off + cw],
                        )
                elif db == DB - 2:
                    # per-chunk DMAs on both rings so the tail pipelines
                    ysb = evict(db)
                    for j, (off, cw) in enumerate(chunk_of_db[db]):
                        eng = nc.sync if (db + j) % 2 == 0 else nc.scalar
                        eng.dma_start(
                            y_d[db * P:(db + 1) * P, off:off + cw],
                            ysb[:, off:off + cw],
                        )
                else:
                    ydma(db, evict(db))

    nc.compile()
    return nc


def _run_spmd(in_maps, C, trace=False, trace_cores=None):
    from concourse.bass_utils import run_bass_kernel_spmd

    nc = _nc_cache.get(C)
    if nc is None:
        nc = _build_bass(C)
        _nc_cache[C] = nc
    return run_bass_kernel_spmd(
        nc,
        in_maps,
        core_ids=list(range(N_CORES)),
        trace=trace,
        trace_cores=trace_cores,
    )


def _prepare(x, weight, bias, ids_sorted):
    """Host-side routing: returns (in_maps, C, counts, starts)."""
    x = np.asarray(x)
    weight = np.asarray(weight)
    bias = np.asarray(bias)
    ids = np.asarray(ids_sorted)

    counts = np.bincount(ids, minlength=N_EXP).astype(np.int64)
    starts = np.zeros(N_EXP, dtype=np.int64)
    starts[1:] = np.cumsum(counts)[:-1]
    C = max(int(counts.max()), 2)
    C += C % 2

    KT = D_IN // P
    DB = D_OUT // P
    xb = x.astype(BF16)
    in_maps = []
    for e in range(N_EXP):
        n_e = int(counts[e])
        xeT = np.zeros((D_IN, C), dtype=BF16)
        if n_e:
            xeT[:, :n_e] = xb[starts[e]:starts[e] + n_e].T
        # partition-flat x: row p, col ki*C + c = x[token c, ki*128+p]
        xf = np.ascontiguousarray(
            xeT.reshape(KT, P, C).transpose(1, 0, 2)
        ).reshape(P, KT * C)
        # flat weight: row p, col db*1024 + kt*128 + d  = W_e[db*128+d, kt*128+p]
        weT = weight[e].T.astype(BF16)  # [d_in, d_out]
        wf = np.ascontiguousarray(
            weT.reshape(KT, P, DB, P).transpose(1, 2, 0, 3)
        ).reshape(P, DB * KT * P)
        bp = np.ascontiguousarray(
            bias[e].astype(BF16).astype(np.float32).reshape(DB, P).T
        )
        in_maps.append({"xf": xf, "wf": wf, "biasp": bp})
    return in_maps, C, counts, starts


def _assemble(results, counts, starts):
    out = np.empty((N_TOK, D_OUT), dtype=BF16)
    for e in range(N_EXP):
        n_e = int(counts[e])
        if n_e:
            out[starts[e]:starts[e] + n_e] = results[e]["yT"][:, :n_e].T
    return out


def kernel(x, weight, bias, ids_sorted):
    in_maps, C, counts, starts = _prepare(x, weight, bias, ids_sorted)
    res = _run_spmd(in_maps, C)
    return _assemble(res.results, counts, starts)
